# revision 1
# baseline (speedup 1.0000x reference)
"""MinHash sketch kernel for Trainium2 (8 NeuronCores, Bass/Tile).

Computes: sketch = segment_min(x @ hash_matrices.T, batch) over 512 segments,
with empty segments set to 0.  x: [N, 256] f32, batch: [N] sorted int64,
hash_matrices: [128, 256] f32 -> out [512, 128] f32.

Strategy (data-parallel over nodes):
  * Host sorts nodes by segment and cuts the order into W=32-wide windows,
    distributed contiguously over the 8 cores (padded with repeats of the
    last node - min-neutral - so every core runs the identical program).
    Windows that straddle a segment boundary (~num_segments of them) are
    recomputed exactly on the host and their device minima ignored.
  * Each core's node shard is laid out TRANSPOSED on host ([256, cols]) so the
    contraction dim (features) sits on SBUF partitions - no on-device
    transpose needed (fp32 DMA transpose doesn't exist and PE transpose would
    make the tensor engine the bottleneck).
  * Device: stream x in 2048-column blocks (descending-width tail so the
    post-final-DMA compute tail is short); hv[h, n] accumulated in PSUM over
    the two 128-feature chunks (h = 128 hashes on partitions), then one
    segmented reduce_min per PSUM bank ([128, 16, 32] -> [128, 16]) into an
    SBUF accumulator [128, G], flushed progressively to DRAM on the SWDGE
    queue so it never blocks the HWDGE input stream.
  * Host: scatter-min each (core, group) column back to its segment, zero
    empty segments.
  * No collective needed: group->segment mapping is host-side, so per-core
    partial sketches are min-combined on the host during unsharding.
  * Cost model (TimelineSim): ~189.1 us/core, DMA-bound at ~358 GB/s HBM
    (64 MB/core input, 0.05% padding); PE ~84% busy, DVE ~42%.

Precision/speed scheme for the matmul (SCHEME):
  * "hilo":  x and H split into bf16 hi+lo pairs on host; 3-term product
             (hi*hi + hi*lo + lo*hi) at full PE rate.  ~4e-6 rel error,
             same DMA bytes as fp32.
  * "f32r":  x, H rounded to FP32R (1-8-11) on host; single-term matmul at
             full PE rate.  ~1.5e-4 rel error.
  * "fp32":  exact fp32 matmul; PE runs at 1/4 rate (2 half-speed passes).
"""

import sys

if "/opt/trn_rl_repo" not in sys.path:
    sys.path.insert(0, "/opt/trn_rl_repo")

import numpy as np

SCHEME = "hilo"
N_CORES = 8
W = 32           # nodes per group (reduce_min granularity)
BANK = 512       # PSUM bank width (fp32)
TB = 2048        # columns per full DMA block
NUM_HASHES = 128
FEATURE_DIM = 256

_compiled_cache = {}


def round_fp32r(a):
    """Round-to-nearest-even to FP32R (1-8-11); low 12 mantissa bits zero."""
    b = np.ascontiguousarray(a, dtype=np.float32).view(np.uint32)
    low = b & np.uint32(0xFFF)
    b2 = b & np.uint32(0xFFFFF000)
    up = (low > 0x800) | ((low == 0x800) & (((b2 >> 12) & 1) == 1))
    return (b2 + (up.astype(np.uint32) << 12)).view(np.float32)


def _build_program(cols, scheme):
    """Build + compile the single-core Bass program for a shard of `cols`
    node-columns (cols % TB == 0)."""
    import concourse.bacc as bacc
    import concourse.mybir as mybir
    import concourse.tile as tile

    nc = bacc.Bacc("TRN2", target_bir_lowering=False, debug=False,
                   num_devices=N_CORES)

    assert cols % W == 0
    n_groups = cols // W
    # full TB-wide blocks, then a descending tail (1024, 512, ..., remainder)
    # so the post-final-DMA compute tail is short
    rest = cols
    block_widths = []
    while rest > 2 * TB:
        block_widths.append(TB)
        rest -= TB
    for piece in (TB, 1024, 512, 512, 512, 512, 512, 512):
        if rest >= piece:
            block_widths.append(piece)
            rest -= piece
        if rest == 0:
            break
    if rest:  # sub-bank remainder (multiple of W) -> partial last PSUM bank
        if block_widths and block_widths[-1] + rest <= TB:
            block_widths[-1] += rest  # merge into last block: one less boundary
        else:
            block_widths.append(rest)
    assert sum(block_widths) == cols, (cols, block_widths)

    if scheme == "hilo":
        xdt = mybir.dt.bfloat16
        x_names = ["xhi", "xlo"]
        h_names = ["hhi", "hlo"]
    else:
        xdt = mybir.dt.float32r if scheme == "f32r" else mybir.dt.float32
        x_names = ["xt"]
        h_names = ["ht"]

    x_in = {n: nc.dram_tensor(n, [FEATURE_DIM, cols], xdt,
                              kind="ExternalInput").ap() for n in x_names}
    h_in = {n: nc.dram_tensor(n, [FEATURE_DIM, NUM_HASHES], xdt,
                              kind="ExternalInput").ap() for n in h_names}
    acc_out = nc.dram_tensor("acc", [NUM_HASHES, n_groups], mybir.dt.float32,
                             kind="ExternalOutput").ap()

    with tile.TileContext(nc) as tc:
        with (
            tc.tile_pool(name="singles", bufs=1) as singles,
            tc.tile_pool(name="xtiles", bufs=3) as xtiles,
            tc.tile_pool(name="psum", bufs=8, space="PSUM") as psum,
        ):
            acc_sb = singles.tile([128, n_groups], mybir.dt.float32)
            h_sb = {}
            for n in h_names:
                t = singles.tile([128, 2, NUM_HASHES], xdt, tag=f"h_{n}")
                # gpsimd queue: don't delay the first x block on the HWDGE queue
                nc.gpsimd.dma_start(out=t[:, 0, :], in_=h_in[n][0:128, :])
                nc.gpsimd.dma_start(out=t[:, 1, :], in_=h_in[n][128:256, :])
                h_sb[n] = t


            # (weight tensor, chunk, rhs tensor) per accumulation term
            if scheme == "hilo":
                phases = [("hhi", 0, "xhi"), ("hhi", 1, "xhi"),
                          ("hlo", 0, "xhi"), ("hlo", 1, "xhi"),
                          ("hhi", 0, "xlo"), ("hhi", 1, "xlo")]
            else:
                phases = [(h_names[0], 0, x_names[0]),
                          (h_names[0], 1, x_names[0])]

            col0 = 0
            flushed = 0
            flush_step = max(TB, cols // 8)
            flush_at = flush_step
            for tb in block_widths:
                sl = slice(col0, col0 + tb)
                bank_widths = [min(BANK, tb - k * BANK)
                               for k in range(-(-tb // BANK))]
                x_sb = {}
                for n in x_names:
                    t = xtiles.tile([128, 2, TB], xdt, tag=f"x_{n}")
                    nc.sync.dma_start(out=t[:, 0, :tb], in_=x_in[n][0:128, sl])
                    nc.sync.dma_start(out=t[:, 1, :tb], in_=x_in[n][128:256, sl])
                    x_sb[n] = t

                hv = []
                for _k in bank_widths:
                    hv_bank = psum.tile([128, BANK], mybir.dt.float32, tag="hv")
                    hv.append(hv_bank)
                for p, (hn, chunk, xn) in enumerate(phases):
                    for k, bw in enumerate(bank_widths):
                        ksl = slice(k * BANK, k * BANK + bw)
                        nc.tensor.matmul(hv[k][:, :bw], h_sb[hn][:, chunk, :],
                                         x_sb[xn][:, chunk, ksl],
                                         start=(p == 0),
                                         stop=(p == len(phases) - 1))

                for k, bw in enumerate(bank_widths):
                    g0 = (col0 + k * BANK) // W
                    nc.vector.tensor_reduce(
                        out=acc_sb[:, g0:g0 + bw // W],
                        in_=hv[k][:, :bw].rearrange("p (g w) -> p g w", w=W),
                        axis=mybir.AxisListType.X,
                        op=mybir.AluOpType.min,
                    )
                col0 += tb
                # flush finished accumulator ranges on the SWDGE queue (doesn't
                # block the HWDGE input stream); keep only the last block's
                # groups for the final flush so the serial tail is tiny
                if col0 >= flush_at and col0 < cols:
                    g1 = col0 // W
                    nc.gpsimd.dma_start(out=acc_out[:, flushed:g1],
                                        in_=acc_sb[:, flushed:g1])
                    flushed = g1
                    flush_at = col0 + flush_step

            # final flush on HWDGE: the input queue is drained by now and
            # HWDGE first-byte latency is ~0.4us lower than SWDGE
            nc.sync.dma_start(out=acc_out[:, flushed:], in_=acc_sb[:, flushed:])

    nc.compile()
    return nc


def kernel(x, batch, num_segments, hash_matrices):
    import ml_dtypes
    from concourse import bass_utils

    x = np.ascontiguousarray(np.asarray(x), dtype=np.float32)
    batch = np.asarray(batch).astype(np.int64).ravel()
    num_segments = int(num_segments)
    hm = np.asarray(hash_matrices, dtype=np.float32)

    assert x.shape[1] == FEATURE_DIM and hm.shape == (NUM_HASHES, FEATURE_DIM)

    # --- host: window construction -----------------------------------------
    # Sort nodes by segment, pad to a uniform per-core column count with
    # repeats of the last node (same segment -> min-neutral), and cut the
    # order into fixed W-wide windows.  A window whose nodes all share one
    # segment is reduced on device; the ~num_segments windows that straddle
    # a segment boundary are recomputed exactly on the host (tiny).
    n_nodes = batch.shape[0]
    counts = np.bincount(batch, minlength=num_segments)
    order = np.argsort(batch, kind="stable")  # contiguous runs per segment

    gpc = -(-(-(-n_nodes // N_CORES)) // W)   # ceil(ceil(n/8)/W)
    cols = gpc * W
    n_pad = cols * N_CORES - n_nodes
    ord_pad = np.concatenate([order, np.full(n_pad, order[-1], dtype=np.int64)])
    idx = ord_pad.reshape(N_CORES, cols)

    bs = batch[ord_pad].reshape(N_CORES, gpc, W)   # sorted segment per slot
    pure = bs[:, :, 0] == bs[:, :, -1]
    grp_seg = np.where(pure, bs[:, :, 0], -1)      # [N_CORES, gpc]

    # --- host: build per-core shards ---------------------------------------
    bf16 = ml_dtypes.bfloat16
    in_maps = []
    if SCHEME == "hilo":
        hhi = hm.T.astype(bf16)
        hlo = (hm.T - hhi.astype(np.float32)).astype(bf16)
        hhi = np.ascontiguousarray(hhi)
        hlo = np.ascontiguousarray(hlo)
        for c in range(N_CORES):
            xt = x[idx[c]].T                         # [256, cols] f32
            xhi = xt.astype(bf16)
            xlo = (xt - xhi.astype(np.float32)).astype(bf16)
            in_maps.append({"xhi": np.ascontiguousarray(xhi),
                            "xlo": np.ascontiguousarray(xlo),
                            "hhi": hhi, "hlo": hlo})
    elif SCHEME == "f32r":
        ht = round_fp32r(np.ascontiguousarray(hm.T))
        for c in range(N_CORES):
            in_maps.append({"xt": round_fp32r(np.ascontiguousarray(x[idx[c]].T)),
                            "ht": ht})
    else:
        ht = np.ascontiguousarray(hm.T)
        for c in range(N_CORES):
            in_maps.append({"xt": np.ascontiguousarray(x[idx[c]].T), "ht": ht})

    # --- device ------------------------------------------------------------
    key = (cols, SCHEME)
    if key not in _compiled_cache:
        _compiled_cache[key] = _build_program(cols, SCHEME)
    nc = _compiled_cache[key]

    res = bass_utils.run_bass_kernel_spmd(
        nc, in_maps, core_ids=list(range(N_CORES)), trace=False
    )

    # --- host: combine -----------------------------------------------------
    sketch = np.full((num_segments, NUM_HASHES), np.inf, dtype=np.float32)
    for c in range(N_CORES):
        acc = res.results[c]["acc"]                 # [128, gpc]
        valid = grp_seg[c] >= 0
        np.minimum.at(sketch, grp_seg[c][valid], acc.T[valid])
    # exact host fixup for boundary (impure) windows
    fix_nodes = idx.reshape(N_CORES, gpc, W)[~pure].ravel()
    if fix_nodes.size:
        hv_fix = x[fix_nodes] @ hm.T               # [n_fix, 128] fp32
        np.minimum.at(sketch, batch[fix_nodes], hv_fix)
    sketch[counts == 0] = 0.0
    return sketch

